# revision 2
# baseline (speedup 1.0000x reference)
"""Trainium2 Bass kernel for LocalGraphProjection — v3.

v3 vs v2 (845us): Vector was 93% busy; everything below targets DVE work.
  - Entry layout [c2, s, cp=2] (channel-pair innermost): weight broadcast
    becomes a MIDDLE stride-0 dim (full 2x f16 rate; inner stride-0 is half
    rate, measured) and every reduction add reads contiguous runs >= 2
    (stride-2 reads are half rate, measured).
  - L0 keeps only the 6 useful window slots (cols 0..2 x rows): 480 of 512
    entry values multiplied instead of 512.
  - idx-first emission per view: projection -> idx -> wrap, THEN weights;
    first gather starts ~10us in instead of ~134us.
  - One 2048-index dma_gather per (tile, view): 48 calls instead of 96.
  - mod-based floors/parities: 1-2 DVE ops instead of 3-5.
  - stats: clamp -> Scalar Relu; mean/std slots accumulated in out tile.
  - weight slots written duplicated ([m,2] cp pairs) straight to w2 quarters.
"""

import numpy as np

import concourse.bass as bass
import concourse.bacc as bacc
import concourse.mybir as mybir
from concourse.tile import TileContext
from concourse.bass_utils import run_bass_kernel_spmd
from concourse import library_config

F32 = mybir.dt.float32
F16 = mybir.dt.float16
I16 = mybir.dt.int16
I32 = mybir.dt.int32
ALU = mybir.AluOpType
ACTF = mybir.ActivationFunctionType

PI = float(np.pi)

N_PTS = 262144
N_CORES = 8
N_CORE_PTS = N_PTS // N_CORES  # 32768
P = 128
M = N_CORE_PTS // P  # 256 slots per partition

MT2 = 16             # points-per-partition per supertile
NT2 = M // MT2       # 16 supertiles
QM = 64              # w2 quarter size (m per quarter)
NQ = M // QM         # 4 quarters

TAB_ROWS = 224 * 112  # 25088
ENT = 512  # f16 per mega-entry (1024B)
N_QUEUES = 4
SQ3 = 1.0 / float(np.sqrt(3.0))


# ----------------------------------------------------------------- host math
def _camera_affines(cameras: np.ndarray):
    """Per-view affine pc = coord @ A_v + b_v, in float64 (mirrors reference)."""
    cams = cameras.astype(np.float64)

    def cm(param):
        theta = param[0] * (PI / 180.0)
        camy = param[3] * np.sin(param[1] * PI / 180.0)
        lens = param[3] * np.cos(param[1] * PI / 180.0)
        camx = lens * np.cos(theta)
        camz = lens * np.sin(theta)
        Zv = np.array([camx, camy, camz])
        Yv = np.array([camy * np.cos(theta + PI), lens, camy * np.sin(theta + PI)])
        Xv = np.cross(Yv, Zv)
        c = np.stack(
            [Xv / np.linalg.norm(Xv), Yv / np.linalg.norm(Yv), Zv / np.linalg.norm(Zv)]
        )
        return c, Zv

    c0, o0 = cm(cams[0])
    M0 = np.linalg.inv(c0.T)
    A, B = [], []
    for v in range(3):
        cv, ov = cm(cams[v])
        A.append(M0 @ cv.T)            # [3,3]
        B.append((o0 - ov) @ cv.T)     # [3]
    return A, B


def _build_affine_plane(cameras: np.ndarray) -> np.ndarray:
    """[P, 40] fp32: per view v, 12 consts at col v*13:
    [a00,a10,a20,b0, -a01,-a11,-a21,-b1, -a02,-a12,-a22,-b2]."""
    A, B = _camera_affines(cameras)
    row = np.zeros(40, np.float32)
    for v in range(3):
        a, b = A[v], B[v]
        base = v * 13
        row[base + 0 : base + 3] = a[:, 0]
        row[base + 3] = b[0]
        row[base + 4 : base + 7] = -a[:, 1]
        row[base + 7] = -b[1]
        row[base + 8 : base + 11] = -a[:, 2]
        row[base + 11] = -b[2]
    return np.tile(row[None, :], (P, 1))


def _build_megatables(img_feat0, img_feat1, img_feat2):
    """Per view: [25088, 512] f16 entries keyed (i0, jp=j0//2).

    Channel-pair interleaved layout (c = 2*c2 + cp):
      [  0: 96] L0 [c2=8,  s=6, cp=2]  s = r*3+cw; rows {i0,i0+1}c, cols {2jp+cw}c
      [ 96:224] L1 [c2=16, s=4, cp=2]  s = r*2+cc; cell (i0//2, jp)
      [224:480] L2 [c2=32, s=4, cp=2]  s = r*2+cc; cell (i0//4, jp//2)
      [480:512] pad
    """
    f0 = np.asarray(img_feat0)
    f1 = np.asarray(img_feat1)
    f2 = np.asarray(img_feat2)
    tabs = {}
    for v in range(3):
        F0, F1, F2 = f0[v], f1[v], f2[v]  # [224,224,16] [112,112,32] [56,56,64]
        # L0
        R = np.stack([np.arange(224), np.minimum(np.arange(224) + 1, 223)], 1)
        C = np.minimum(2 * np.arange(112)[:, None] + np.arange(3)[None, :], 223)
        t = F0[R]                 # [224, 2, 224, 16]
        t = t[:, :, C]            # [224, 2, 112, 3, 16]
        t = t.reshape(224, 2, 112, 3, 8, 2)
        l0 = np.transpose(t, (0, 2, 4, 1, 3, 5)).reshape(224, 112, 96)
        # L1
        R1 = np.stack([np.arange(112), np.minimum(np.arange(112) + 1, 111)], 1)
        C1 = np.stack([np.arange(112), np.minimum(np.arange(112) + 1, 111)], 1)
        t = F1[R1]                # [112, 2, 112, 32]
        t = t[:, :, C1]           # [112, 2, 112, 2, 32]
        t = t.reshape(112, 2, 112, 2, 16, 2)
        l1 = np.transpose(t, (0, 2, 4, 1, 3, 5)).reshape(112, 112, 128)
        l1 = np.repeat(l1, 2, axis=0)  # [224, 112, 128]
        # L2
        R2 = np.stack([np.arange(56), np.minimum(np.arange(56) + 1, 55)], 1)
        C2 = np.stack([np.arange(56), np.minimum(np.arange(56) + 1, 55)], 1)
        t = F2[R2]                # [56, 2, 56, 64]
        t = t[:, :, C2]           # [56, 2, 56, 2, 64]
        t = t.reshape(56, 2, 56, 2, 32, 2)
        l2 = np.transpose(t, (0, 2, 4, 1, 3, 5)).reshape(56, 56, 256)
        l2 = np.repeat(np.repeat(l2, 4, axis=0), 2, axis=1)  # [224, 112, 256]
        pad = np.zeros((224, 112, 32), np.float32)
        tab = np.concatenate([l0, l1, l2, pad], axis=2)  # [224, 112, 512]
        tabs[v] = np.ascontiguousarray(tab.reshape(TAB_ROWS, ENT).astype(np.float16))
    return tabs


# ------------------------------------------------------------- device kernel
def emit_body(nc, tc, pools, dram):
    V = nc.vector
    G = nc.gpsimd
    S = nc.scalar
    IO = nc.sync

    sc, gpool, fpool, opool, ipool, wpool = (
        pools["sc"], pools["g"], pools["f"], pools["o"], pools["i"], pools["w"],
    )
    coords_d, afp_d, tabs_d, out_d = (
        dram["coords"], dram["afp"], dram["tabs"], dram["out"],
    )

    G.load_library(library_config.mlp)
    nidx_reg = G.alloc_register("nidx")
    G.reg_mov(nidx_reg, P * (MT2 // 2))

    # ---- preload
    coords_sb = sc.tile([P, 3, M], F32, tag="coords", name="coords_sb")
    IO.dma_start(out=coords_sb[:, :, :], in_=coords_d[:, :, :])
    afp = sc.tile([P, 40], F32, tag="afp", name="afp_sb")
    IO.dma_start(out=afp[:, :], in_=afp_d[:, :])

    def ap_s(col):  # [P,1] scalar AP
        return afp[:, col : col + 1]

    cx = coords_sb[:, 0, :]
    cy = coords_sb[:, 1, :]
    cz = coords_sb[:, 2, :]

    def newt(tag, dt=F32, pool=sc):
        return pool.tile([P, M], dt, tag=tag, name=tag)

    # saved per-view tiles for phase 2 (f16: i0/j0 are exact integers <= 223,
    # fx/fy only feed f16 weight math)
    sv = {}
    for v in range(3):
        for nm in ("fx", "fy", "i0", "j0"):
            sv[(nm, v)] = sc.tile([P, M], F16, tag=f"sv_{nm}{v}", name=f"sv_{nm}{v}")

    # idx wrap plumbing (per-view tiles rotate through the g-pool slots)
    wr_all = ipool.tile([P, 3, 8 * M], I16, tag="wr", name="wr")
    wr6 = wr_all.rearrange("p j (t m k r2) -> p j t m k r2", m=MT2 // 2, k=4, r2=2)

    # ---------------- phase 1 per view: projection -> idx -> wrap
    for v in range(3):
        base = v * 13
        X = newt("ph1_X")
        nY = newt("ph1_nY")
        nZ = newt("ph1_nZ")
        for out_t, off in ((X, 0), (nY, 4), (nZ, 8)):
            S.activation(out_t[:, :], cx, ACTF.Identity,
                         scale=ap_s(base + off + 0), bias=ap_s(base + off + 3))
            V.scalar_tensor_tensor(
                out_t[:, :], cy, ap_s(base + off + 1), out_t[:, :], ALU.mult, ALU.add
            )
            V.scalar_tensor_tensor(
                out_t[:, :], cz, ap_s(base + off + 2), out_t[:, :], ALU.mult, ALU.add
            )
        rz = newt("ph1_rz")
        V.reciprocal(rz[:, :], nZ[:, :])
        h = newt("ph1_h")
        w_ = newt("ph1_w")
        # scratch-tag reuse (liveness): nY dead after h, nZ after rz,
        # X/rz after w_, h after i0, w_ after j0
        V.tensor_tensor(h[:, :], nY[:, :], rz[:, :], ALU.mult)
        V.tensor_scalar(h[:, :], h[:, :], 248.0, 112.0, ALU.mult, ALU.add)
        V.tensor_scalar(h[:, :], h[:, :], 0.0, 223.0, ALU.max, ALU.min)
        V.tensor_tensor(w_[:, :], X[:, :], rz[:, :], ALU.mult)
        V.tensor_scalar(w_[:, :], w_[:, :], 248.0, 112.0, ALU.mult, ALU.add)
        V.tensor_scalar(w_[:, :], w_[:, :], 0.0, 223.0, ALU.max, ALU.min)

        fx = newt("ph1_nY")
        i0 = newt("ph1_nZ")
        fy = newt("ph1_rz")
        j0 = newt("ph1_X")
        pj2 = newt("ph1_h")
        xi = newt("xi32", dt=I16)
        for (xx, x1x, fxx) in ((h, i0, fx), (w_, j0, fy)):
            V.tensor_copy(xi[:, :], xx[:, :])
            V.tensor_copy(x1x[:, :], xi[:, :])
            V.tensor_tensor(fxx[:, :], x1x[:, :], xx[:, :], ALU.is_gt)
            V.tensor_tensor(x1x[:, :], x1x[:, :], fxx[:, :], ALU.subtract)
            V.tensor_tensor(fxx[:, :], xx[:, :], x1x[:, :], ALU.subtract)
        V.tensor_copy(xi[:, :], j0[:, :])
        V.tensor_scalar(xi[:, :], xi[:, :], 1, None, ALU.bitwise_and)
        V.tensor_copy(pj2[:, :], xi[:, :])
        for srcT, nm in ((fx, "fx"), (fy, "fy"), (i0, "i0"), (j0, "j0")):
            S.activation(sv[(nm, v)][:, :], srcT[:, :], ACTF.Copy)

        # idx = i0*112 + (j0 - pj2)/2 = (i0*224 + j0 - pj2) * 0.5
        tmp = newt("ph1_w")  # w_ dead after j0
        V.scalar_tensor_tensor(tmp[:, :], i0[:, :], 224.0, j0[:, :], ALU.mult, ALU.add)
        V.tensor_tensor(tmp[:, :], tmp[:, :], pj2[:, :], ALU.subtract)
        V.tensor_scalar(tmp[:, :], tmp[:, :], 0.5, None, ALU.mult)
        idx_v = gpool.tile([P, M], I16, tag="g", name=f"idx{v}", bufs=2)
        i32a = gpool.tile([32, M, 4], I16, tag="g", name=f"i32a{v}", bufs=2)
        i16b = gpool.tile([16, M, 4], I16, tag="g", name=f"i16b{v}", bufs=2)
        V.tensor_copy(idx_v[:, :], tmp[:, :])
        for r4 in range(4):
            V.tensor_copy(i32a[:, :, r4], idx_v[32 * r4 : 32 * (r4 + 1), :])
        IO.dma_start(out=i16b[:, :, :], in_=i32a[16:32, :, :])
        V.tensor_copy(
            wr6[0:16, v, :, :, :, 0],
            i32a[0:16, :, :].rearrange("q (t m) k -> q t m k", m=MT2 // 2),
        )
        V.tensor_copy(
            wr6[0:16, v, :, :, :, 1],
            i16b[:, :, :].rearrange("q (t m) k -> q t m k", m=MT2 // 2),
        )
        IO.dma_start(out=wr_all[16:32, v, :], in_=wr_all[0:16, v, :])
        IO.dma_start(out=wr_all[32:64, v, :], in_=wr_all[0:32, v, :])
        IO.dma_start(out=wr_all[64:128, v, :], in_=wr_all[0:64, v, :])

    # ---------------- phase 2 per view: fracs + products (f16, shared scratch)
    w2 = wpool.tile([P, 3, M, 28], F16, tag="w2", name="w2")

    def newt16(tag):
        return sc.tile([P, M], F16, tag=tag, name=tag)

    for v in range(3):
        fx, fy = sv[("fx", v)], sv[("fy", v)]
        i0, j0 = sv[("i0", v)], sv[("j0", v)]
        wx1 = newt16("w_wx1")
        wy1 = newt16("w_wy1")
        V.scalar_tensor_tensor(wx1[:, :], fx[:, :], 0.0, fx[:, :], ALU.is_gt, ALU.subtract)
        V.scalar_tensor_tensor(wy1[:, :], fy[:, :], 0.0, fy[:, :], ALU.is_gt, ALU.subtract)

        # L0 col weights: a0 = wy1*(1-pj2); a1 = wy1*pj2 + fy*(1-pj2); a2 = fy*pj2
        xi = newt("xi32", dt=I16)

        def parity(dst, srcT, msk):
            V.tensor_copy(xi[:, :], srcT[:, :])
            V.tensor_scalar(xi[:, :], xi[:, :], msk, None, ALU.bitwise_and)
            V.tensor_copy(dst[:, :], xi[:, :])

        pj2 = newt16("w_par")
        parity(pj2, j0, 1)
        t1 = newt16("w_fx1")  # t1 dead before fx1 is written
        a2 = newt16("w_a2")
        a0 = newt16("w_a0")
        a1 = newt16("w_a1")
        V.tensor_tensor(t1[:, :], wy1[:, :], pj2[:, :], ALU.mult)
        V.tensor_tensor(a0[:, :], wy1[:, :], t1[:, :], ALU.subtract)
        V.tensor_tensor(a2[:, :], fy[:, :], pj2[:, :], ALU.mult)
        V.tensor_tensor(a1[:, :], t1[:, :], fy[:, :], ALU.add)
        V.tensor_tensor(a1[:, :], a1[:, :], a2[:, :], ALU.subtract)

        # level fracs + row weights (parity scratch rotates through 2 tags)
        fx1 = newt16("w_fx1")
        fy1 = newt16("w_fy1")
        fx2 = newt16("w_fx2")
        fy2 = newt16("w_fy2")
        V.tensor_tensor(fy1[:, :], fy[:, :], pj2[:, :], ALU.add)
        V.tensor_scalar(fy1[:, :], fy1[:, :], 0.5, None, ALU.mult)
        pj4 = newt16("w_par")
        parity(pj4, j0, 3)
        V.tensor_tensor(fy2[:, :], fy[:, :], pj4[:, :], ALU.add)
        V.tensor_scalar(fy2[:, :], fy2[:, :], 0.25, None, ALU.mult)
        pi2 = newt16("w_par")
        parity(pi2, i0, 1)
        V.tensor_tensor(fx1[:, :], fx[:, :], pi2[:, :], ALU.add)
        V.tensor_scalar(fx1[:, :], fx1[:, :], 0.5, None, ALU.mult)
        pi4 = newt16("w_par")
        parity(pi4, i0, 3)
        V.tensor_tensor(fx2[:, :], fx[:, :], pi4[:, :], ALU.add)
        V.tensor_scalar(fx2[:, :], fx2[:, :], 0.25, None, ALU.mult)
        wx11 = newt16("w_wx11")
        wy11 = newt16("w_wy11")
        wx12 = newt16("w_wx12")
        wy12 = newt16("w_wy12")
        for (f, o) in ((fx1, wx11), (fy1, wy11), (fx2, wx12), (fy2, wy12)):
            V.scalar_tensor_tensor(o[:, :], f[:, :], 0.0, f[:, :], ALU.is_gt, ALU.subtract)

        pairs = [
            (wx1, a0), (wx1, a1), (wx1, a2),
            (fx, a0), (fx, a1), (fx, a2),
            (wx11, wy11), (wx11, fy1), (fx1, wy11), (fx1, fy1),
            (wx12, wy12), (wx12, fy2), (fx2, wy12), (fx2, fy2),
        ]
        wc = sc.tile([P, 14, M], F16, tag="wc", name="wc")
        for s, (rw, cw) in enumerate(pairs):
            V.tensor_tensor(wc[:, s, :], rw[:, :], cw[:, :], ALU.mult)
        S.activation(
            w2[:, v, :, :].rearrange("p m (s two) -> p m s two", two=2),
            wc[:, :, :].rearrange("p s m -> p m s").unsqueeze(3)
            .broadcast_to([P, M, 14, 2]),
            ACTF.Copy,
        )

    # ---------------- supertile loop
    gq = [0]
    for t in range(NT2):
        lm = slice(t * MT2, (t + 1) * MT2)

        gt = gpool.tile([P, 3, MT2, ENT], F16, tag="g", name="g", bufs=2)
        F_t = fpool.tile([P, 3, MT2, 112], F16, tag="F", name="F")
        for v in range(3):
            hm8 = MT2 // 2
            for hh in range(2):
                G.dma_gather(
                    gt[:, v, hh * hm8 : (hh + 1) * hm8, :],
                    tabs_d[v][:, :],
                    wr_all[:, v, (2 * t + hh) * 8 * hm8 : (2 * t + hh + 1) * 8 * hm8],
                    P * hm8,
                    nidx_reg,
                    ENT,
                    queue_num=gq[0] % N_QUEUES,
                )
                gq[0] += 1

            gv = gt[:, v, :, :]
            a = gv[:, :, 0:96].rearrange("p m (c s) -> p m c s", s=12)
            b = gv[:, :, 96:224].rearrange("p m (c s) -> p m c s", s=8)
            c = gv[:, :, 224:480].rearrange("p m (c s) -> p m c s", s=8)
            V.tensor_tensor(
                a, a,
                w2[:, v, lm, 0:12].unsqueeze(2).broadcast_to([P, MT2, 8, 12]),
                ALU.mult)
            V.tensor_tensor(
                b, b,
                w2[:, v, lm, 12:20].unsqueeze(2).broadcast_to([P, MT2, 16, 8]),
                ALU.mult)
            V.tensor_tensor(
                c, c,
                w2[:, v, lm, 20:28].unsqueeze(2).broadcast_to([P, MT2, 32, 8]),
                ALU.mult)
            # reductions (all contiguous runs >= 2)
            V.tensor_tensor(a[:, :, :, 0:6], a[:, :, :, 0:6], a[:, :, :, 6:12], ALU.add)
            V.tensor_tensor(a[:, :, :, 0:2], a[:, :, :, 0:2], a[:, :, :, 2:4], ALU.add)
            F_l0 = F_t[:, v, :, 0:16].rearrange("p m (c two) -> p m c two", two=2)
            V.tensor_tensor(F_l0, a[:, :, :, 0:2], a[:, :, :, 4:6], ALU.add)
            V.tensor_tensor(b[:, :, :, 0:4], b[:, :, :, 0:4], b[:, :, :, 4:8], ALU.add)
            F_l1 = F_t[:, v, :, 16:48].rearrange("p m (c two) -> p m c two", two=2)
            V.tensor_tensor(F_l1, b[:, :, :, 0:2], b[:, :, :, 2:4], ALU.add)
            V.tensor_tensor(c[:, :, :, 0:4], c[:, :, :, 0:4], c[:, :, :, 4:8], ALU.add)
            F_l2 = F_t[:, v, :, 48:112].rearrange("p m (c two) -> p m c two", two=2)
            V.tensor_tensor(F_l2, c[:, :, :, 0:2], c[:, :, :, 2:4], ALU.add)

        # ---- stats across views, per half-tile
        for hh in range(2):
            hm = MT2 // 2
            hs = slice(hh * hm, (hh + 1) * hm)
            sl = slice(t * MT2 + hh * hm, t * MT2 + (hh + 1) * hm)
            out_t = opool.tile([P, hm, 336], F16, tag="out", name="out_t")
            F0 = F_t[:, 0, hs, :]
            F1 = F_t[:, 1, hs, :]
            F2 = F_t[:, 2, hs, :]
            fmax = out_t[:, :, 0:112]
            fmean = out_t[:, :, 112:224]
            fstd = out_t[:, :, 224:336]
            sqa = fpool.tile([P, hm, 112], F16, tag="sqa", name="sqa")
            V.tensor_tensor(fmax, F0, F1, ALU.max)
            V.tensor_tensor(fmax, fmax, F2, ALU.max)
            V.tensor_tensor(sqa[:, :, :], F0, F1, ALU.add)
            V.tensor_tensor(sqa[:, :, :], sqa[:, :, :], F2, ALU.add)
            S.activation(fmean, sqa[:, :, :], ACTF.Identity, scale=1.0 / 3.0)
            S.activation(fstd, F0, ACTF.Square, scale=SQ3)
            S.activation(sqa[:, :, :], F1, ACTF.Square, scale=SQ3)
            V.tensor_tensor(fstd, fstd, sqa[:, :, :], ALU.add)
            S.activation(sqa[:, :, :], F2, ACTF.Square, scale=SQ3)
            V.tensor_tensor(fstd, fstd, sqa[:, :, :], ALU.add)
            S.activation(sqa[:, :, :], fmean, ACTF.Square)
            V.tensor_tensor(fstd, fstd, sqa[:, :, :], ALU.subtract)
            S.activation(sqa[:, :, :], fstd, ACTF.Relu)
            S.activation(fstd, sqa[:, :, :], ACTF.Sqrt)

            IO.dma_start(out=out_d[:, sl, :], in_=out_t[:, :, :])


def build_kernel():
    nc = bacc.Bacc("TRN2", num_swdge_queues=N_QUEUES, dynamic_dma_scratch_size=8192)
    coords = nc.dram_tensor("coords", [P, 3, M], F32, kind="ExternalInput")
    afp = nc.dram_tensor("afp", [P, 40], F32, kind="ExternalInput")
    tabs = {}
    for v in range(3):
        tabs[v] = nc.dram_tensor(f"tab{v}", [TAB_ROWS, ENT], F16, kind="ExternalInput")
    out = nc.dram_tensor("out", [P, M, 336], F16, kind="ExternalOutput")

    with nc.allow_low_precision("fp16 sampling kernel"), TileContext(nc) as tc:
        import contextlib

        stack = contextlib.ExitStack()
        pools = {
            "sc": stack.enter_context(tc.tile_pool(name="sc", bufs=1)),
            "w": stack.enter_context(tc.tile_pool(name="w", bufs=1)),
            "g": stack.enter_context(tc.tile_pool(name="g", bufs=2)),
            "f": stack.enter_context(tc.tile_pool(name="f", bufs=2)),
            "o": stack.enter_context(tc.tile_pool(name="o", bufs=2)),
            "i": stack.enter_context(tc.tile_pool(name="i", bufs=1)),
        }
        dram = {
            "coords": coords.ap(),
            "afp": afp.ap(),
            "tabs": {v: t.ap() for v, t in tabs.items()},
            "out": out.ap(),
        }
        with stack:
            emit_body(nc, tc, pools, dram)
    nc.compile()
    return nc


# ------------------------------------------------------------------ frontend
_NC_CACHE = {}
TRACE = False
LAST_RES = [None]


def _get_nc():
    if "nc" not in _NC_CACHE:
        _NC_CACHE["nc"] = build_kernel()
    return _NC_CACHE["nc"]


def kernel(coord, img_feat0, img_feat1, img_feat2, cameras):
    coord = np.asarray(coord, np.float32)
    afp = _build_affine_plane(np.asarray(cameras, np.float32))
    tabs = _build_megatables(img_feat0, img_feat1, img_feat2)

    nc = _get_nc()
    in_maps = []
    for k in range(N_CORES):
        shard = coord[k * N_CORE_PTS : (k + 1) * N_CORE_PTS]  # [32768, 3]
        cs = np.ascontiguousarray(
            shard.reshape(P, M, 3).transpose(0, 2, 1)
        )  # [P, 3, M]
        im = {"coords": cs, "afp": afp}
        for v in range(3):
            im[f"tab{v}"] = tabs[v]
        in_maps.append(im)

    res = run_bass_kernel_spmd(
        nc, in_maps, core_ids=list(range(N_CORES)), trace=TRACE
    )
    LAST_RES[0] = res
    stats = np.concatenate(
        [res.results[k]["out"].reshape(N_CORE_PTS, 336) for k in range(N_CORES)], 0
    ).astype(np.float32)
    return np.concatenate([coord, stats], axis=1)
